# revision 32
# baseline (speedup 1.0000x reference)
"""AR-LSTM sampling kernel for 8 TRN2 NeuronCores.

nn_ARLSTMModel: 1024-step autoregressive LSTM rollout (H=512, D=64, bs=256),
data-parallel over batch (32 rows/core), weights replicated in SBUF.

Math (per step, per core, feature-major):
    gates = W~ @ h + W_ih @ z + b~          (W~ = W_hh + W_ih@W_pmu folds the
                                             mu-part of the y feedback;
                                             z = eps*std is the only feedback)
    i,f,o ~ sigmoid, g ~ tanh; sigmoid computed as 0.5*tanh(x/2)+0.5 so the
    whole step uses one ACT table set (tanh+exp).
    State is stored doubled (H=2h, C=2c) so the cell update becomes fused
    scalar_tensor_tensor ops:
        A  = (tanh_i + 1) * tanh_g
        B  = (tanh_f + 1) * C
        C' = 0.5*B + A
        H' = (tanh_o + 1) * tanh(0.5*C')
    with the 0.5 factors folded into the weights host-side.
    proj: [mu|lv] = 0.5*W_proj @ H + b_proj (bias via const `1` row in the
    moving state vector s4 = [z(64); 1; 0...]).
    y = z + mu,  z' = eps_t * exp(0.5*lv)
"""

import numpy as np

H, D, BS, NT = 512, 64, 256, 1024
NCORES = 8
B = BS // NCORES  # 32
P = 128
U = 16  # steps per For_i iteration


# ---------------------------------------------------------------- weights fold
def fold_weights(W_ih, W_hh, b_ih, b_hh, W_proj, b_proj):
    """Host-side fold + layout. Returns dict of fp16 arrays for DRAM params."""
    W_ih = np.asarray(W_ih, np.float32)
    W_hh = np.asarray(W_hh, np.float32)
    W_proj = np.asarray(W_proj, np.float32)
    b_proj = np.asarray(b_proj, np.float32)
    W_pmu = W_proj[:D]          # [64, 512]
    b_mu = b_proj[:D]
    Wt = W_hh + W_ih @ W_pmu    # [2048, 512]
    bt = np.asarray(b_ih, np.float32) + np.asarray(b_hh, np.float32) + W_ih @ b_mu

    # m-tile order is gate-type-major: m = 4j + q with banks [g, i, f, o],
    # so PSUM bank j holds one whole gate [128 rows x 4 chunks] and the cell
    # update runs as full-width [128,128] fused ops.
    goff = {0: 2 * H, 1: 0, 2: H, 3: 3 * H}    # g, i, f, o
    wh = np.zeros((P, 4, 16, P), np.float16)   # [K=128, kc, m, M=128]
    w4 = np.zeros((P, 16, P), np.float16)      # [K=128 (z,1,pad), m, M]
    for j in range(4):
        for q in range(4):
            m = 4 * j + q
            rows = slice(goff[j] + P * q, goff[j] + P * q + P)
            s = 1.0 if j == 0 else 0.5         # sigmoid-as-tanh prescale
            for kc in range(4):
                # extra 0.5: state stored as H=2h
                wh[:, kc, m, :] = (s * 0.5 * Wt[rows, P * kc:P * (kc + 1)]).T
            w4[:D, m, :] = (s * W_ih[rows, :]).T   # z enters unscaled
            w4[D, m, :] = s * bt[rows]             # bias row (hits the `1`)
    wp = np.zeros((P, 4, P), np.float16)       # proj moving [K=H chunk, kc, 2D]
    for kc in range(4):
        wp[:, kc, :] = (0.5 * W_proj[:, P * kc:P * (kc + 1)]).T
    bp = np.zeros((P, P), np.float16)          # proj bias rhs, row D = b_proj
    bp[D, :] = b_proj
    return {"wh": wh, "w4": w4, "wp": wp, "bp": bp}


# ---------------------------------------------------------------- bass builder
def build_nc(nt=NT, u=U, debug=False, n_dummy=3):
    import concourse.mybir as mybir
    import concourse.tile as tile
    from concourse import bacc
    from concourse.bass import ds
    from concourse.masks import make_identity
    from contextlib import ExitStack

    f32 = mybir.dt.float32
    f16 = mybir.dt.float16
    Tanh = mybir.ActivationFunctionType.Tanh
    Exp = mybir.ActivationFunctionType.Exp
    add = mybir.AluOpType.add
    mult = mybir.AluOpType.mult
    subtract = mybir.AluOpType.subtract

    assert nt % u == 0

    nc = bacc.Bacc("TRN2")
    h0_d = nc.declare_dram_parameter("h0", [B, H], f32, isOutput=False)
    c0_d = nc.declare_dram_parameter("c0", [B, H], f32, isOutput=False)
    y0_d = nc.declare_dram_parameter("y0", [B, D], f32, isOutput=False)
    eps_d = nc.declare_dram_parameter("eps", [B, nt, D], f32, isOutput=False)
    wh_d = nc.declare_dram_parameter("wh", [P, 4, 16, P], f16, isOutput=False)
    w4_d = nc.declare_dram_parameter("w4", [P, 16, P], f16, isOutput=False)
    wp_d = nc.declare_dram_parameter("wp", [P, 4, P], f16, isOutput=False)
    bp_d = nc.declare_dram_parameter("bp", [P, P], f16, isOutput=False)
    ys_d = nc.declare_dram_parameter("ys", [B, nt, D], f32, isOutput=True)
    mus_d = nc.declare_dram_parameter("mus", [B, nt, D], f32, isOutput=True)
    lvs_d = nc.declare_dram_parameter("lvs", [B, nt, D], f32, isOutput=True)
    if debug:
        dbg_gt = nc.declare_dram_parameter("dbg_gt", [P, 4 * P], f16, isOutput=True)
        dbg_h = nc.declare_dram_parameter("dbg_h", [P, P], f16, isOutput=True)
        dbg_c = nc.declare_dram_parameter("dbg_c", [P, P], f32, isOutput=True)
        dbg_s4 = nc.declare_dram_parameter("dbg_s4", [P, 32], f16, isOutput=True)
        dbg_h0 = nc.declare_dram_parameter("dbg_h0", [P, P], f16, isOutput=True)

    with tile.TileContext(nc) as tc, ExitStack() as ctx:
        const = ctx.enter_context(tc.tile_pool(name="const", bufs=1))
        state = ctx.enter_context(tc.tile_pool(name="state", bufs=1))
        dyn = ctx.enter_context(tc.tile_pool(name="dyn", bufs=2))
        tmp = ctx.enter_context(tc.tile_pool(name="tmp", bufs=3))
        # PSUM bank budget (8): psg 4 gate banks + 1 warm-up, psp 2, pst 1.
        psg_pool = ctx.enter_context(tc.tile_pool(name="psg", bufs=1, space="PSUM"))
        psp_pool = ctx.enter_context(tc.tile_pool(name="psp", bufs=2, space="PSUM"))
        pst_pool = ctx.enter_context(tc.tile_pool(name="pst", bufs=1, space="PSUM"))

        # ---- constants
        wh_sb = const.tile([P, 4, 16, P], f16)
        nc.sync.dma_start(wh_sb, wh_d[:])
        w4_sb = const.tile([P, 16, P], f16)
        nc.sync.dma_start(w4_sb, w4_d[:])
        wp_sb = const.tile([P, 4, P], f16)
        nc.sync.dma_start(wp_sb, wp_d[:])
        bp_sb = const.tile([P, P], f16)
        nc.sync.dma_start(bp_sb, bp_d[:])
        ident = const.tile([32, 32], f32)
        make_identity(nc, ident)

        # ---- state (feature-major; free dim 128 = 4 H-chunks of 32 batch)
        hT = state.tile([P, P], f16)       # H = 2h
        cT = state.tile([P, P], f32)       # C = 2c
        s4 = state.tile([P, 32], f16)      # [z(64); 1; 0(63)] moving chunk
        gt = state.tile([P, 4 * P], f16)   # tanh(gates): [g | i | f | o]
        tc_sb = state.tile([P, P], f16)    # tanh(c)
        zf_st = state.tile([B, D], f32)    # z of the previous step (batch-major)
        # gate PSUM: [g|i] share a bank; f and o get their own banks so the
        # f-side cell update can run while PE writes the o bank.
        psg_gi = psg_pool.tile([P, 2 * P], f32, tag="psg_gi")
        psg_f = psg_pool.tile([P, P], f32, tag="psg_f")
        psg_o = psg_pool.tile([P, P], f32, tag="psg_o")
        warm = psg_pool.tile([P, 512], f32, tag="warm")  # HAM-warmer target
        tc_ps = psg_pool.tile([P, P], f32, tag="tc")     # tanh(c) (ScE->PSUM)

        # ---- prologue: load & transpose initial state, z0 = y0 - mu(h0)
        h0_sb = tmp.tile([B, H], f32, tag="init_h")
        nc.sync.dma_start(h0_sb, h0_d[:])
        c0_sb = tmp.tile([B, H], f32, tag="init_c")
        nc.sync.dma_start(c0_sb, c0_d[:])
        y0_sb = tmp.tile([B, D], f32, tag="init_y")
        nc.sync.dma_start(y0_sb, y0_d[:])

        nc.vector.memset(s4, 0.0)
        nc.vector.memset(s4[D:D + 1, :], 1.0)

        # Warm-up transpose consuming only `ident`: the PE vector clock then
        # covers the gpsimd tick, so the h0/c0 transposes below carry a
        # single sync wait each (walrus's LDWEIGHTS struct holds only one).
        nc.tensor.transpose(warm[0:32, 0:32], ident, ident)

        for q in range(4):
            pt = pst_pool.tile([P, 32], f32, tag="ztr")
            nc.tensor.transpose(pt, h0_sb[:, P * q:P * (q + 1)], ident)
            nc.vector.tensor_scalar_mul(hT[:, 32 * q:32 * (q + 1)], pt, 2.0)
            pt2 = pst_pool.tile([P, 32], f32, tag="ztr")
            nc.tensor.transpose(pt2, c0_sb[:, P * q:P * (q + 1)], ident)
            nc.vector.tensor_scalar_mul(cT[:, 32 * q:32 * (q + 1)], pt2, 2.0)

        ps_p0 = psp_pool.tile([B, P], f32, tag="psp")
        for kc in range(4):
            nc.tensor.matmul(ps_p0, hT[:, 32 * kc:32 * (kc + 1)], wp_sb[:, kc, :],
                             start=(kc == 0), stop=False)
        nc.tensor.matmul(ps_p0, s4, bp_sb, start=False, stop=True)
        # z0 = y0 - mu(h0); transposed into s4 inside step 0's sweep
        nc.vector.tensor_tensor(zf_st, y0_sb, ps_p0[:, 0:D], subtract)
        if debug:
            nc.sync.dma_start(dbg_h0[:], hT)

        # ---- main loop
        with tc.For_i(0, nt, u) as iv:
            eps_sb = dyn.tile([B, u, D], f32, tag="eps")
            nc.sync.dma_start(eps_sb, eps_d[:, ds(iv, u), :])
            y_st = dyn.tile([B, u, D], f32, tag="yst")
            mu_st = dyn.tile([B, u, D], f32, tag="must")
            lv_st = dyn.tile([B, u, D], f32, tag="lvst")

            for tt in range(u):
                # Gate banks (gate-type-major m = 4j+q, banks [g,i] | [f] | [o]).
                # Order: h-MMs for g,i,f; transpose last step's z into s4;
                # z-MMs for g,i (then tanh_gi, A); z-MMs for f (tanh_f, B, C',
                # tanh_c run while PE does the o bank); o h+z MMs; tanh_o; H.
                # Dummy matmuls into `warm` fill the PE gap before proj so the
                # HAM clock gate stays at 2.4 GHz.
                def bank_ap(j, q):
                    if j == 0:
                        return psg_gi[:, 32 * q:32 * (q + 1)]
                    if j == 1:
                        return psg_gi[:, P + 32 * q:P + 32 * (q + 1)]
                    if j == 2:
                        return psg_f[:, 32 * q:32 * (q + 1)]
                    return psg_o[:, 32 * q:32 * (q + 1)]

                def h_mms(j, start):
                    for q in range(4):
                        for kc in range(4):
                            nc.tensor.matmul(bank_ap(j, q),
                                             wh_sb[:, kc, 4 * j + q, :],
                                             hT[:, 32 * kc:32 * (kc + 1)],
                                             start=(start and q == 0 and kc == 0),
                                             stop=False)

                def z_mms(j, stop):
                    for q in range(4):
                        nc.tensor.matmul(bank_ap(j, q), w4_sb[:, 4 * j + q, :],
                                         s4, start=False,
                                         stop=(stop and q == 3))

                h_mms(2, True)   # f first: B=(tf+1)*C drops off the chain
                h_mms(0, True)
                h_mms(1, False)
                # transpose z_{t-1} into s4 (mid-sweep: zf_st long ready)
                ztr = pst_pool.tile([D, 32], f32, tag="ztr")
                nc.tensor.transpose(ztr, zf_st, ident)
                nc.vector.tensor_copy(s4[0:D, :], ztr)
                z_mms(2, True)
                nc.scalar.activation(gt[:, 2 * P:3 * P], psg_f, Tanh)
                Bt = tmp.tile([P, P], f32, tag="B")
                nc.vector.scalar_tensor_tensor(          # B = (tf+1)*C
                    Bt, gt[:, 2 * P:3 * P], 1.0, cT, add, mult)
                z_mms(0, False)
                z_mms(1, True)
                nc.scalar.activation(gt[:, 0:2 * P], psg_gi, Tanh)
                A = tmp.tile([P, P], f32, tag="A")
                nc.vector.scalar_tensor_tensor(          # A = (ti+1)*tg
                    A, gt[:, P:2 * P], 1.0, gt[:, 0:P], add, mult)
                nc.vector.scalar_tensor_tensor(          # C' = 0.5B + A
                    cT, Bt, 0.5, A, mult, add)
                h_mms(3, True)
                z_mms(3, True)
                nc.scalar.activation(gt[:, 3 * P:4 * P], psg_o, Tanh)
                nc.scalar.activation(tc_ps, cT, Tanh, scale=0.5)
                nc.vector.scalar_tensor_tensor(          # H = (to+1)*tanh_c
                    hT, gt[:, 3 * P:4 * P], 1.0, tc_ps, add, mult)

                # HAM warmers: junk matmuls with no step dependencies that
                # keep the PE busy while ACT/DVE finish the cell update.
                for _ in range(n_dummy):
                    nc.tensor.matmul(warm, wh_sb[:, 0, 0, :],
                                     wh_sb[:, 0, 0:4, :],
                                     start=True, stop=True)

                # proj (batch-major out): [mu|lv] psum [32, 128]
                ps_p = psp_pool.tile([B, P], f32, tag="psp")
                for kc in range(4):
                    nc.tensor.matmul(ps_p, hT[:, 32 * kc:32 * (kc + 1)],
                                     wp_sb[:, kc, :],
                                     start=(kc == 0), stop=False)
                nc.tensor.matmul(ps_p, s4, bp_sb, start=False, stop=True)

                # z path (zf_st overwrite is WAR-ordered after this step's ztr)
                std = tmp.tile([B, D], f32, tag="std")
                nc.scalar.activation(std, ps_p[:, D:2 * D], Exp, scale=0.5)
                nc.vector.tensor_tensor(zf_st, eps_sb[:, tt, :], std, mult)
                nc.vector.tensor_tensor(y_st[:, tt, :], zf_st, ps_p[:, 0:D], add)
                nc.vector.tensor_copy(mu_st[:, tt, :], ps_p[:, 0:D])
                nc.vector.tensor_copy(lv_st[:, tt, :], ps_p[:, D:2 * D])

            if debug:
                nc.sync.dma_start(dbg_gt[:], gt)
                nc.sync.dma_start(dbg_h[:], hT)
                nc.sync.dma_start(dbg_c[:], cT)
                nc.sync.dma_start(dbg_s4[:], s4)
            nc.sync.dma_start(ys_d[:, ds(iv, u), :], y_st)
            nc.sync.dma_start(mus_d[:, ds(iv, u), :], mu_st)
            nc.sync.dma_start(lvs_d[:, ds(iv, u), :], lv_st)

    if not nc.is_finalized():
        nc.finalize()
    return nc


# ---------------------------------------------------------------- entry point
_cache = {}


def run_kernel(h0, c0, yt, eps, W_ih, W_hh, b_ih, b_hh, W_proj, b_proj,
               trace=False, **spmd_kwargs):
    """Build (cached), shard, execute on 8 cores; returns (outputs, results)."""
    from concourse.bass_utils import run_bass_kernel_spmd

    h0 = np.asarray(h0, np.float32)
    c0 = np.asarray(c0, np.float32)
    yt = np.asarray(yt, np.float32)
    eps = np.asarray(eps, np.float32)

    if "nc" not in _cache:
        _cache["nc"] = build_nc(NT, U)
    nc = _cache["nc"]
    wd = fold_weights(W_ih, W_hh, b_ih, b_hh, W_proj, b_proj)

    in_maps = []
    for c in range(NCORES):
        sl = slice(B * c, B * (c + 1))
        in_maps.append({
            "h0": np.ascontiguousarray(h0[sl]),
            "c0": np.ascontiguousarray(c0[sl]),
            "y0": np.ascontiguousarray(yt[sl, 0, :]),
            "eps": np.ascontiguousarray(eps[sl]),
            **wd,
        })

    res = run_bass_kernel_spmd(nc, in_maps, core_ids=list(range(NCORES)),
                               trace=trace, **spmd_kwargs)
    ys = np.concatenate([res.results[c]["ys"] for c in range(NCORES)], axis=0)
    mus = np.concatenate([res.results[c]["mus"] for c in range(NCORES)], axis=0)
    lvs = np.concatenate([res.results[c]["lvs"] for c in range(NCORES)], axis=0)
    return (ys, mus, lvs), res


def kernel(input=None, h0=None, c0=None, yt=None, eps=None, W_ih=None,
           W_hh=None, b_ih=None, b_hh=None, W_proj=None, b_proj=None,
           **kwargs):
    out, _ = run_kernel(h0, c0, yt, eps, W_ih, W_hh, b_ih, b_hh,
                        W_proj, b_proj)
    return out


# revision 35
# speedup vs baseline: 1.2518x; 1.2518x over previous
"""AR-LSTM sampling kernel for 8 TRN2 NeuronCores.

nn_ARLSTMModel: 1024-step autoregressive LSTM rollout (H=512, D=64, bs=256),
data-parallel over batch (32 rows/core), weights replicated in SBUF.

Math (per step, per core, feature-major):
    gates = W~ @ h + W_ih @ z + b~          (W~ = W_hh + W_ih@W_pmu folds the
                                             mu-part of the y feedback;
                                             z = eps*std is the only feedback)
    i,f,o ~ sigmoid, g ~ tanh; sigmoid computed as 0.5*tanh(x/2)+0.5 so the
    whole step uses one ACT table set (tanh+exp).
    State is stored doubled (H=2h, C=2c) so the cell update becomes fused
    scalar_tensor_tensor ops:
        A  = (tanh_i + 1) * tanh_g
        B  = (tanh_f + 1) * C
        C' = 0.5*B + A
        H' = (tanh_o + 1) * tanh(0.5*C')
    with the 0.5 factors folded into the weights host-side.
    proj: [mu|lv] = 0.5*W_proj @ H + b_proj (bias via const `1` row in the
    moving state vector s4 = [z(64); 1; 0...]).
    y = z + mu,  z' = eps_t * exp(0.5*lv)
"""

import numpy as np

H, D, BS, NT = 512, 64, 256, 1024
NCORES = 8
B = BS // NCORES  # 32
P = 128
U = 32  # steps per For_i iteration


# ---------------------------------------------------------------- weights fold
def fold_weights(W_ih, W_hh, b_ih, b_hh, W_proj, b_proj):
    """Host-side fold + layout. Returns dict of fp16 arrays for DRAM params."""
    W_ih = np.asarray(W_ih, np.float32)
    W_hh = np.asarray(W_hh, np.float32)
    W_proj = np.asarray(W_proj, np.float32)
    b_proj = np.asarray(b_proj, np.float32)
    W_pmu = W_proj[:D]          # [64, 512]
    b_mu = b_proj[:D]
    Wt = W_hh + W_ih @ W_pmu    # [2048, 512]
    bt = np.asarray(b_ih, np.float32) + np.asarray(b_hh, np.float32) + W_ih @ b_mu

    # m-tile order is gate-type-major: m = 4j + q with banks [g, i, f, o],
    # so PSUM bank j holds one whole gate [128 rows x 4 chunks] and the cell
    # update runs as full-width [128,128] fused ops.
    goff = {0: 2 * H, 1: 0, 2: H, 3: 3 * H}    # g, i, f, o
    wh = np.zeros((P, 4, 16, P), np.float16)   # [K=128, kc, m, M=128]
    w4 = np.zeros((P, 16, P), np.float16)      # [K=128 (z,1,pad), m, M]
    for j in range(4):
        for q in range(4):
            m = 4 * j + q
            rows = slice(goff[j] + P * q, goff[j] + P * q + P)
            s = 1.0 if j == 0 else 0.5         # sigmoid-as-tanh prescale
            for kc in range(4):
                # extra 0.5: state stored as H=2h
                wh[:, kc, m, :] = (s * 0.5 * Wt[rows, P * kc:P * (kc + 1)]).T
            w4[:D, m, :] = (s * W_ih[rows, :]).T   # z enters unscaled
            w4[D, m, :] = s * bt[rows]             # bias row (hits the `1`)
    wp = np.zeros((P, 4, P), np.float16)       # proj moving [K=H chunk, kc, 2D]
    for kc in range(4):
        wp[:, kc, :] = (0.5 * W_proj[:, P * kc:P * (kc + 1)]).T
    bp = np.zeros((P, P), np.float16)          # proj bias rhs, row D = b_proj
    bp[D, :] = b_proj
    return {"wh": wh, "w4": w4, "wp": wp, "bp": bp}


# ---------------------------------------------------------------- bass builder
def build_nc(nt=NT, u=U, debug=False, n_dummy=0):
    import concourse.mybir as mybir
    import concourse.tile as tile
    from concourse import bacc
    from concourse.bass import ds
    from concourse.masks import make_identity
    from contextlib import ExitStack

    f32 = mybir.dt.float32
    f16 = mybir.dt.float16
    Tanh = mybir.ActivationFunctionType.Tanh
    Exp = mybir.ActivationFunctionType.Exp
    add = mybir.AluOpType.add
    mult = mybir.AluOpType.mult
    subtract = mybir.AluOpType.subtract

    assert nt % u == 0

    nc = bacc.Bacc("TRN2")
    h0_d = nc.declare_dram_parameter("h0", [B, H], f32, isOutput=False)
    c0_d = nc.declare_dram_parameter("c0", [B, H], f32, isOutput=False)
    y0_d = nc.declare_dram_parameter("y0", [B, D], f32, isOutput=False)
    eps_d = nc.declare_dram_parameter("eps", [B, nt, D], f32, isOutput=False)
    wh_d = nc.declare_dram_parameter("wh", [P, 4, 16, P], f16, isOutput=False)
    w4_d = nc.declare_dram_parameter("w4", [P, 16, P], f16, isOutput=False)
    wp_d = nc.declare_dram_parameter("wp", [P, 4, P], f16, isOutput=False)
    bp_d = nc.declare_dram_parameter("bp", [P, P], f16, isOutput=False)
    ys_d = nc.declare_dram_parameter("ys", [B, nt, D], f32, isOutput=True)
    mus_d = nc.declare_dram_parameter("mus", [B, nt, D], f32, isOutput=True)
    lvs_d = nc.declare_dram_parameter("lvs", [B, nt, D], f32, isOutput=True)
    if debug:
        dbg_gt = nc.declare_dram_parameter("dbg_gt", [P, 4 * P], f16, isOutput=True)
        dbg_h = nc.declare_dram_parameter("dbg_h", [P, P], f16, isOutput=True)
        dbg_c = nc.declare_dram_parameter("dbg_c", [P, P], f32, isOutput=True)
        dbg_s4 = nc.declare_dram_parameter("dbg_s4", [P, 32], f16, isOutput=True)
        dbg_h0 = nc.declare_dram_parameter("dbg_h0", [P, P], f16, isOutput=True)

    with tile.TileContext(nc) as tc, ExitStack() as ctx:
        const = ctx.enter_context(tc.tile_pool(name="const", bufs=1))
        state = ctx.enter_context(tc.tile_pool(name="state", bufs=1))
        dyn = ctx.enter_context(tc.tile_pool(name="dyn", bufs=2))
        tmp = ctx.enter_context(tc.tile_pool(name="tmp", bufs=3))
        # PSUM bank budget (8): psg 4 gate banks + 1 warm-up, psp 2, pst 1.
        psg_pool = ctx.enter_context(tc.tile_pool(name="psg", bufs=1, space="PSUM"))
        psp_pool = ctx.enter_context(tc.tile_pool(name="psp", bufs=2, space="PSUM"))
        pst_pool = ctx.enter_context(tc.tile_pool(name="pst", bufs=1, space="PSUM"))

        # ---- constants
        wh_sb = const.tile([P, 4, 16, P], f16)
        nc.sync.dma_start(wh_sb, wh_d[:])
        w4_sb = const.tile([P, 16, P], f16)
        nc.sync.dma_start(w4_sb, w4_d[:])
        wp_sb = const.tile([P, 4, P], f16)
        nc.sync.dma_start(wp_sb, wp_d[:])
        bp_sb = const.tile([P, P], f16)
        nc.sync.dma_start(bp_sb, bp_d[:])
        ident = const.tile([32, 32], f32)
        make_identity(nc, ident)

        # ---- state (feature-major; free dim 128 = 4 H-chunks of 32 batch)
        hT = state.tile([P, P], f16)       # H = 2h
        cT = state.tile([P, P], f32)       # C = 2c
        s4 = state.tile([P, 32], f16)      # [z(64); 1; 0(63)] moving chunk
        gt = state.tile([P, 4 * P], f16)   # tanh(gates): [g | i | f | o]
        tc_sb = state.tile([P, P], f16)    # tanh(c)
        zf_st = state.tile([B, D], f32)    # z of the previous step (batch-major)
        # gate PSUM: [g|i] share a bank; f and o get their own banks so the
        # f-side cell update can run while PE writes the o bank.
        psg_gi = psg_pool.tile([P, 2 * P], f32, tag="psg_gi")
        psg_f = psg_pool.tile([P, P], f32, tag="psg_f")
        psg_o = psg_pool.tile([P, P], f32, tag="psg_o")
        warm = psg_pool.tile([P, 512], f32, tag="warm")  # HAM-warmer target
        tc_ps = psg_pool.tile([P, P], f32, tag="tc")     # tanh(c) (ScE->PSUM)

        # ---- prologue: load & transpose initial state, z0 = y0 - mu(h0)
        h0_sb = tmp.tile([B, H], f32, tag="init_h")
        nc.sync.dma_start(h0_sb, h0_d[:])
        c0_sb = tmp.tile([B, H], f32, tag="init_c")
        nc.sync.dma_start(c0_sb, c0_d[:])
        y0_sb = tmp.tile([B, D], f32, tag="init_y")
        nc.sync.dma_start(y0_sb, y0_d[:])

        nc.vector.memset(s4, 0.0)
        nc.vector.memset(s4[D:D + 1, :], 1.0)

        # Warm-up transpose consuming only `ident`: the PE vector clock then
        # covers the gpsimd tick, so the h0/c0 transposes below carry a
        # single sync wait each (walrus's LDWEIGHTS struct holds only one).
        nc.tensor.transpose(warm[0:32, 0:32], ident, ident)

        for q in range(4):
            pt = pst_pool.tile([P, 32], f32, tag="ztr")
            nc.tensor.transpose(pt, h0_sb[:, P * q:P * (q + 1)], ident)
            nc.vector.tensor_scalar_mul(hT[:, 32 * q:32 * (q + 1)], pt, 2.0)
            pt2 = pst_pool.tile([P, 32], f32, tag="ztr")
            nc.tensor.transpose(pt2, c0_sb[:, P * q:P * (q + 1)], ident)
            nc.vector.tensor_scalar_mul(cT[:, 32 * q:32 * (q + 1)], pt2, 2.0)

        ps_p0 = psp_pool.tile([B, P], f32, tag="psp")
        for kc in range(4):
            nc.tensor.matmul(ps_p0, hT[:, 32 * kc:32 * (kc + 1)], wp_sb[:, kc, :],
                             start=(kc == 0), stop=False)
        nc.tensor.matmul(ps_p0, s4, bp_sb, start=False, stop=True)
        # z0 = y0 - mu(h0); transposed into s4 inside step 0's sweep
        nc.vector.tensor_tensor(zf_st, y0_sb, ps_p0[:, 0:D], subtract)
        if debug:
            nc.sync.dma_start(dbg_h0[:], hT)

        # ---- main loop (PE body spans several IRAM blocks: hint the branch)
        with tc.For_i(0, nt, u, hint_engines=(mybir.EngineType.PE,)) as iv:
            eps_sb = dyn.tile([B, u, D], f32, tag="eps")
            nc.sync.dma_start(eps_sb, eps_d[:, ds(iv, u), :])
            y_st = dyn.tile([B, u, D], f32, tag="yst")
            mu_st = dyn.tile([B, u, D], f32, tag="must")
            lv_st = dyn.tile([B, u, D], f32, tag="lvst")

            for tt in range(u):
                # Gate banks (gate-type-major m = 4j+q, banks [g,i] | [f] | [o]).
                # Order: h-MMs for g,i,f; transpose last step's z into s4;
                # z-MMs for g,i (then tanh_gi, A); z-MMs for f (tanh_f, B, C',
                # tanh_c run while PE does the o bank); o h+z MMs; tanh_o; H.
                # Dummy matmuls into `warm` fill the PE gap before proj so the
                # HAM clock gate stays at 2.4 GHz.
                def bank_ap(j, q):
                    if j == 0:
                        return psg_gi[:, 32 * q:32 * (q + 1)]
                    if j == 1:
                        return psg_gi[:, P + 32 * q:P + 32 * (q + 1)]
                    if j == 2:
                        return psg_f[:, 32 * q:32 * (q + 1)]
                    return psg_o[:, 32 * q:32 * (q + 1)]

                def h_mms(j, start):
                    for q in range(4):
                        for kc in range(4):
                            nc.tensor.matmul(bank_ap(j, q),
                                             wh_sb[:, kc, 4 * j + q, :],
                                             hT[:, 32 * kc:32 * (kc + 1)],
                                             start=(start and q == 0 and kc == 0),
                                             stop=False)

                def z_mms(j, stop):
                    for q in range(4):
                        nc.tensor.matmul(bank_ap(j, q), w4_sb[:, 4 * j + q, :],
                                         s4, start=False,
                                         stop=(stop and q == 3))

                h_mms(2, True)   # f first: B=(tf+1)*C drops off the chain
                h_mms(0, True)
                h_mms(1, False)
                # transpose z_{t-1} into s4 (mid-sweep: zf_st long ready)
                ztr = pst_pool.tile([D, 32], f32, tag="ztr")
                nc.tensor.transpose(ztr, zf_st, ident)
                nc.vector.tensor_copy(s4[0:D, :], ztr)
                z_mms(2, True)
                nc.scalar.activation(gt[:, 2 * P:3 * P], psg_f, Tanh)
                Bt = tmp.tile([P, P], f32, tag="B")
                nc.vector.scalar_tensor_tensor(          # B = (tf+1)*C
                    Bt, gt[:, 2 * P:3 * P], 1.0, cT, add, mult)
                z_mms(0, False)
                z_mms(1, True)
                nc.scalar.activation(gt[:, 0:2 * P], psg_gi, Tanh)
                A = tmp.tile([P, P], f32, tag="A")
                nc.vector.scalar_tensor_tensor(          # A = (ti+1)*tg
                    A, gt[:, P:2 * P], 1.0, gt[:, 0:P], add, mult)
                nc.vector.scalar_tensor_tensor(          # C' = 0.5B + A
                    cT, Bt, 0.5, A, mult, add)
                h_mms(3, True)
                z_mms(3, True)
                nc.scalar.activation(gt[:, 3 * P:4 * P], psg_o, Tanh)
                nc.scalar.activation(tc_ps, cT, Tanh, scale=0.5)
                nc.vector.scalar_tensor_tensor(          # H = (to+1)*tanh_c
                    hT, gt[:, 3 * P:4 * P], 1.0, tc_ps, add, mult)

                # HAM warmers: junk matmuls with no step dependencies that
                # keep the PE busy while ACT/DVE finish the cell update.
                for _ in range(n_dummy):
                    nc.tensor.matmul(warm, wh_sb[:, 0, 0, :],
                                     wh_sb[:, 0, 0:4, :],
                                     start=True, stop=True)

                # proj (batch-major out): [mu|lv] psum [32, 128]
                ps_p = psp_pool.tile([B, P], f32, tag="psp")
                for kc in range(4):
                    nc.tensor.matmul(ps_p, hT[:, 32 * kc:32 * (kc + 1)],
                                     wp_sb[:, kc, :],
                                     start=(kc == 0), stop=False)
                nc.tensor.matmul(ps_p, s4, bp_sb, start=False, stop=True)

                # z path (zf_st overwrite is WAR-ordered after this step's ztr)
                std = tmp.tile([B, D], f32, tag="std")
                nc.scalar.activation(std, ps_p[:, D:2 * D], Exp, scale=0.5)
                nc.vector.tensor_tensor(zf_st, eps_sb[:, tt, :], std, mult)
                nc.vector.tensor_tensor(y_st[:, tt, :], zf_st, ps_p[:, 0:D], add)
                nc.vector.tensor_copy(mu_st[:, tt, :], ps_p[:, 0:D])
                nc.vector.tensor_copy(lv_st[:, tt, :], ps_p[:, D:2 * D])

            if debug:
                nc.sync.dma_start(dbg_gt[:], gt)
                nc.sync.dma_start(dbg_h[:], hT)
                nc.sync.dma_start(dbg_c[:], cT)
                nc.sync.dma_start(dbg_s4[:], s4)
            nc.sync.dma_start(ys_d[:, ds(iv, u), :], y_st)
            nc.sync.dma_start(mus_d[:, ds(iv, u), :], mu_st)
            nc.sync.dma_start(lvs_d[:, ds(iv, u), :], lv_st)

    if not nc.is_finalized():
        nc.finalize()
    return nc


# ---------------------------------------------------------------- entry point
_cache = {}


def run_kernel(h0, c0, yt, eps, W_ih, W_hh, b_ih, b_hh, W_proj, b_proj,
               trace=False, **spmd_kwargs):
    """Build (cached), shard, execute on 8 cores; returns (outputs, results)."""
    from concourse.bass_utils import run_bass_kernel_spmd

    h0 = np.asarray(h0, np.float32)
    c0 = np.asarray(c0, np.float32)
    yt = np.asarray(yt, np.float32)
    eps = np.asarray(eps, np.float32)

    if "nc" not in _cache:
        _cache["nc"] = build_nc(NT, U)
    nc = _cache["nc"]
    wd = fold_weights(W_ih, W_hh, b_ih, b_hh, W_proj, b_proj)

    in_maps = []
    for c in range(NCORES):
        sl = slice(B * c, B * (c + 1))
        in_maps.append({
            "h0": np.ascontiguousarray(h0[sl]),
            "c0": np.ascontiguousarray(c0[sl]),
            "y0": np.ascontiguousarray(yt[sl, 0, :]),
            "eps": np.ascontiguousarray(eps[sl]),
            **wd,
        })

    res = run_bass_kernel_spmd(nc, in_maps, core_ids=list(range(NCORES)),
                               trace=trace, **spmd_kwargs)
    ys = np.concatenate([res.results[c]["ys"] for c in range(NCORES)], axis=0)
    mus = np.concatenate([res.results[c]["mus"] for c in range(NCORES)], axis=0)
    lvs = np.concatenate([res.results[c]["lvs"] for c in range(NCORES)], axis=0)
    return (ys, mus, lvs), res


def kernel(input=None, h0=None, c0=None, yt=None, eps=None, W_ih=None,
           W_hh=None, b_ih=None, b_hh=None, W_proj=None, b_proj=None,
           **kwargs):
    out, _ = run_kernel(h0, c0, yt, eps, W_ih, W_hh, b_ih, b_hh,
                        W_proj, b_proj)
    return out
